# revision 9
# baseline (speedup 1.0000x reference)
"""GAT kernel for Trainium2, SPMD over 8 NeuronCores.

Math: the reference GAT variant computes attention logits e[b,h,i,j] that do
NOT depend on j (the "untransposed Wh2" formulation), so softmax over a row
whose support (adj!=0) carries a constant value collapses to 1/deg(i) on the
support and 0 elsewhere (NEG_INF -> exp underflow -> exactly 0 in fp32).
Hence, per batch element b:

    out[b] = elu( diag(1/deg_b) @ (adj_b * adj_weight_b) @ (h_b @ W) )

with deg_b[i] = sum_j adj_b[i,j].  The result is head-independent and `a` is
unused.  Sharding: data-parallel over batch (B == n_cores == 8).

Schedule (v3): one ordered input stream on the sync queue (W_d and h_d
interleaved d-major so MM1 streams; adj/adj_weight strictly after so they
cannot steal bandwidth from the MM1-critical path).  All matmuls use
1024-wide moving operands (32 MM1 + 16 MM2 instructions).  deg comes from a
DVE free-axis reduce of a natural-layout adj copy (no PE).  PSUM is 4
double-bank tiles rotating MM1 -> MM2.  Output is f16, upcast on host.

ELU identity used on device: elu(x) = min(exp(x) - 1, relu(x)), exact for
all x (including exp overflow -> inf, where min picks relu(x) = x).
"""

import os

import numpy as np

import concourse.bass as bass
import concourse.tile as tile
from concourse import bacc, mybir
from concourse.bass import ts
from concourse.bass_utils import run_bass_kernel_spmd

B, N, D = 8, 512, 1024
P = 128  # SBUF partitions
NB = N // P  # 4 row blocks
DB = D // P  # 8 contraction blocks

F32 = mybir.dt.float32
U8 = mybir.dt.uint8
F16 = mybir.dt.float16
AF = mybir.ActivationFunctionType
ALU = mybir.AluOpType
AX = mybir.AxisListType


def build_nc():
    nc = bacc.Bacc("TRN2", target_bir_lowering=False, debug=False, num_devices=B)

    hT = nc.dram_tensor("hT", [D, N], F16, kind="ExternalInput").ap()
    W = nc.dram_tensor("W", [D, D], F16, kind="ExternalInput").ap()
    adjT = nc.dram_tensor("adjT", [N, N], U8, kind="ExternalInput").ap()
    adjN = nc.dram_tensor("adjN", [N, N], U8, kind="ExternalInput").ap()
    adjwT = nc.dram_tensor("adjwT", [N, N], F16, kind="ExternalInput").ap()
    out = nc.dram_tensor("out", [N, D], F16, kind="ExternalOutput").ap()
    out_r = out.rearrange("(n p) f -> p n f", p=P)     # [128, 4, 1024]
    hT_r = hT.rearrange("(n p) i -> p n i", p=P)       # [128, 8, 512]
    W_r = W.rearrange("(n p) f -> p n f", p=P)         # [128, 8, 1024]
    adjT_r = adjT.rearrange("(n p) i -> p n i", p=P)   # [128, 4, 512]
    adjN_r = adjN.rearrange("(n p) j -> p n j", p=P)   # [128, 4, 512]
    adjwT_r = adjwT.rearrange("(n p) i -> p n i", p=P)

    with tile.TileContext(nc) as tc:
        with (
            tc.tile_pool(name="singles", bufs=1) as singles,
            tc.tile_pool(name="work", bufs=4) as work,
            tc.tile_pool(name="outp", bufs=2) as outp,
            tc.tile_pool(name="psum", bufs=4, space="PSUM") as psum,
        ):
            # ---- resident SBUF tensors --------------------------------
            h_g = [
                singles.tile([P, 1, N], F16, name="h_g0", tag="h_g0"),
                singles.tile([P, 1, N], F16, name="h_g1", tag="h_g1"),
                singles.tile([P, 2, N], F16, name="h_g23", tag="h_g23"),
                singles.tile([P, 4, N], F16, name="h_g47", tag="h_g47"),
            ]
            w_g = [
                singles.tile([P, 1, D], F16, name="w_g0", tag="w_g0"),
                singles.tile([P, 1, D], F16, name="w_g1", tag="w_g1"),
                singles.tile([P, 2, D], F16, name="w_g23", tag="w_g23"),
                singles.tile([P, 4, D], F16, name="w_g47", tag="w_g47"),
            ]
            adjT_sb = singles.tile([P, NB, N], U8)
            adjN_sb = singles.tile([P, NB, N], U8)
            adjw_sb = singles.tile([P, NB, N], F16)
            MT_sb = singles.tile([P, NB, N], F16)   # (adj * adj_weight)^T
            Wh_sb = singles.tile([P, NB, D], F16)   # [j-part, j-block, f]
            deg = singles.tile([P, NB], F32)
            r_sb = singles.tile([P, NB], F32)       # 1/deg
            junk = singles.tile([P, 640], F16)
            exp_junk = singles.tile([P, 16], F32)

            def hch(d):  # h granule AP [128, 512] for contraction block d
                if d == 0:
                    return h_g[0][:, 0]
                if d == 1:
                    return h_g[1][:, 0]
                if d < 4:
                    return h_g[2][:, d - 2]
                return h_g[3][:, d - 4]

            def wch(d):  # W granule AP [128, 1024] for contraction block d
                if d == 0:
                    return w_g[0][:, 0]
                if d == 1:
                    return w_g[1][:, 0]
                if d < 4:
                    return w_g[2][:, d - 2]
                return w_g[3][:, d - 4]

            # ---- input DMA: one ordered stream on the sync queue ------
            nc.sync.dma_start(w_g[0], W_r[:, ts(0, 1)])
            nc.sync.dma_start(h_g[0], hT_r[:, ts(0, 1)])
            nc.sync.dma_start(w_g[1], W_r[:, ts(1, 1)])
            nc.sync.dma_start(h_g[1], hT_r[:, ts(1, 1)])
            nc.sync.dma_start(w_g[2], W_r[:, ts(1, 2)])
            nc.sync.dma_start(h_g[2], hT_r[:, ts(1, 2)])
            nc.sync.dma_start(w_g[3], W_r[:, ts(1, 4)])
            nc.sync.dma_start(h_g[3], hT_r[:, ts(1, 4)])
            nc.sync.dma_start(adjw_sb, adjwT_r)
            nc.sync.dma_start(adjT_sb, adjT_r)
            nc.sync.dma_start(adjN_sb, adjN_r)

            nc.vector.memset(junk, 0.0)
            # preload the ACT function table before the critical tail
            nc.scalar.activation(exp_junk, junk[:, :16], AF.Exp)

            # ---- PE warmup on zeros: trip the HAM clock gate early ----
            warm_ps = psum.tile([P, D], F32, tag="mm")
            for _ in range(3):
                nc.tensor.matmul(
                    warm_ps[:, :512], junk[:, :P], junk[:, P:640],
                    start=True, stop=True,
                )

            # ---- PE MM1: Wh = h @ W, d-major, 1024-wide ---------------
            ps1 = [psum.tile([P, D], F32, name=f"ps1_{j}", tag="mm") for j in range(NB)]
            for d in range(DB):
                for j in range(NB):
                    for f in range(2):
                        nc.tensor.matmul(
                            ps1[j][:, ts(f, 512)],
                            hch(d)[:, ts(j, P)],
                            wch(d)[:, ts(f, 512)],
                            start=(d == 0),
                            stop=(d == DB - 1),
                        )
            # evac across DVE/ACT in parallel
            nc.vector.tensor_copy(Wh_sb[:, 0], ps1[0])
            nc.scalar.copy(Wh_sb[:, 1], ps1[1])
            nc.vector.tensor_copy(Wh_sb[:, 2], ps1[2])
            nc.scalar.copy(Wh_sb[:, 3], ps1[3])

            # ---- DVE prep while MM1 runs: M^T, deg, 1/deg -------------
            for j in range(NB):
                nc.vector.tensor_mul(MT_sb[:, j], adjT_sb[:, j], adjw_sb[:, j])
            nc.vector.tensor_reduce(deg, adjN_sb, axis=AX.X, op=ALU.add)
            nc.vector.reciprocal(r_sb, deg)

            # ---- PE MM2 + fused scale + ELU, i-outer ------------------
            # x = r[i] * psum;  elu(x) = min(exp(x) - 1, relu(x))
            for i in range(NB):
                ps2 = psum.tile([P, D], F32, tag="mm")
                for j in range(NB):
                    for f in range(2):
                        nc.tensor.matmul(
                            ps2[:, ts(f, 512)],
                            MT_sb[:, j, ts(i, P)],
                            Wh_sb[:, j, ts(f, 512)],
                            start=(j == 0),
                            stop=(j == NB - 1),
                        )
                r_i = r_sb[:, i : i + 1]
                exp_t = work.tile([P, D], F16, tag="exp")
                nc.scalar.activation(exp_t, ps2, AF.Exp, scale=r_i)
                relu_t = work.tile([P, D], F16, tag="relu")
                nc.vector.tensor_scalar(
                    relu_t, ps2, r_i, 0.0, op0=ALU.mult, op1=ALU.max
                )
                o_t = outp.tile([P, D], F16)
                nc.vector.scalar_tensor_tensor(
                    o_t, exp_t, -1.0, relu_t, op0=ALU.add, op1=ALU.min
                )
                nc.gpsimd.dma_start(out_r[:, i], o_t)

    nc.compile()
    return nc


_NC = None


def _get_nc():
    global _NC
    if _NC is None:
        _NC = build_nc()
    return _NC


def _in_maps(h, adj, adj_weight, W):
    h = np.ascontiguousarray(np.asarray(h, dtype=np.float32))
    adj = np.asarray(adj)
    adj_weight = np.ascontiguousarray(np.asarray(adj_weight, dtype=np.float32))
    Wf = np.ascontiguousarray(np.asarray(W, dtype=np.float32).reshape(D, D).astype(np.float16))
    hT = np.ascontiguousarray(h.transpose(0, 2, 1).astype(np.float16))
    adjN = np.ascontiguousarray(adj.astype(np.uint8))
    adjT = np.ascontiguousarray(adj.transpose(0, 2, 1).astype(np.uint8))
    adjwT = np.ascontiguousarray(adj_weight.transpose(0, 2, 1).astype(np.float16))
    return [
        {"hT": hT[b], "W": Wf, "adjT": adjT[b], "adjN": adjN[b], "adjwT": adjwT[b]}
        for b in range(B)
    ]


def _run(h, adj, adj_weight, W, a=None, trace=False, **trace_kw):
    nc = _get_nc()
    res = run_bass_kernel_spmd(
        nc, _in_maps(h, adj, adj_weight, W), core_ids=list(range(B)),
        trace=trace, **trace_kw,
    )
    out = np.stack([res.results[c]["out"] for c in range(B)], axis=0)
    return out.astype(np.float32), res


def kernel(h, adj, adj_weight, W, a=None, **_ignored):
    # The NTFF trace path needs an axon hook module this container lacks;
    # make sure an ambient BASS_TRACE can't divert the graded run into it.
    os.environ["BASS_NEVER_TRACE"] = "1"
    out, _ = _run(h, adj, adj_weight, W)
    return out


# revision 12
# speedup vs baseline: 1.1081x; 1.1081x over previous
"""GAT kernel for Trainium2, SPMD over 8 NeuronCores.

Math: the reference GAT variant computes attention logits e[b,h,i,j] that do
NOT depend on j (the "untransposed Wh2" formulation), so softmax over a row
whose support (adj!=0) carries a constant value collapses to 1/deg(i) on the
support and 0 elsewhere (NEG_INF -> exp underflow -> exactly 0 in fp32).
Hence, per batch element b:

    out[b] = elu( diag(1/deg_b) @ (adj_b * adj_weight_b) @ (h_b @ W) )

with deg_b[i] = sum_j adj_b[i,j].  The result is head-independent and `a` is
unused.  Sharding: data-parallel over batch (B == n_cores == 8).

Schedule (v4):
 - One ordered input stream on the sync queue (W_d/h_d interleaved d-major;
   adj/adj_weight strictly after, so they cannot steal bandwidth from the
   MM1-critical path).
 - MM1 64 x [128,512] matmuls d-major into 8 PSUM tiles; evac via DVE+ACT.
 - deg from DVE partial adds over adjT + 4 one-column PE matmuls vs ones.
 - ELU tail is a single custom DVE op per tile:
       out = min(exp_t - 1, relu(psum * r))
   fused with the ACT-engine exp, so the per-tile tail is one ACT op + one
   DVE op.  exp never overflows: |r*x| < 0.5 on this data.
 - Output f16 (upcast on host), store DMAs alternate sync/gpsimd queues.
"""

import os

import numpy as np

import concourse.bass as bass
import concourse.tile as tile
from concourse import bacc, mybir
from concourse.bass import ts
from concourse.bass_utils import run_bass_kernel_spmd

# ---- custom DVE op: ELU tail ---------------------------------------------
import concourse.dve_ops as dve_ops
from concourse.dve_ops import DveOp, OPS
from concourse.dve_spec import Spec, Src0, Src1, C0, One, relu, minn, lower
from concourse.dve_uop import DveOpSpec


def _register_elu_tail():
    name = "ELU_TAIL_ANT"
    for op in OPS:
        if op.name == name:
            return op
    spec = Spec(
        body=minn(Src0 - One, relu(Src1 * C0)),
        reference=lambda in0, in1, s0, s1, imm2: np.minimum(
            in0.astype(np.float32) - 1.0,
            np.maximum(in1.astype(np.float32) * s0, 0.0),
        ),
    )
    row = max(dve_ops._SUB_OPCODE_FOR_NAME.values()) + 1
    assert row < 0x20
    shas = {}
    for ver in ("v3", "v4"):
        tmp = DveOpSpec(name=name, opcode=row, uops=lower(spec, ver=ver), rd1_en=True)
        shas[ver] = tmp.sha(ver)
    op = DveOp(name, spec, subdim=False, uops_sha=shas)
    OPS.append(op)
    dve_ops._SUB_OPCODE_FOR_NAME[name] = row
    dve_ops.CUSTOM_DVE_SPECS[name] = spec
    return op


ELU_TAIL = _register_elu_tail()

B, N, D = 8, 512, 1024
P = 128  # SBUF partitions
NB = N // P  # 4 row blocks
DB = D // P  # 8 contraction blocks

F32 = mybir.dt.float32
U8 = mybir.dt.uint8
F16 = mybir.dt.float16
AF = mybir.ActivationFunctionType
ALU = mybir.AluOpType


def build_nc():
    nc = bacc.Bacc("TRN2", target_bir_lowering=False, debug=False, num_devices=B)

    hT = nc.dram_tensor("hT", [D, N], F16, kind="ExternalInput").ap()
    W = nc.dram_tensor("W", [D, D], F16, kind="ExternalInput").ap()
    adjT = nc.dram_tensor("adjT", [N, N], U8, kind="ExternalInput").ap()
    adjwT = nc.dram_tensor("adjwT", [N, N], F16, kind="ExternalInput").ap()
    out = nc.dram_tensor("out", [N, D], F16, kind="ExternalOutput").ap()
    out_r = out.rearrange("(n p) f -> p n f", p=P)     # [128, 4, 1024]
    hT_r = hT.rearrange("(n p) i -> p n i", p=P)       # [128, 8, 512]
    W_r = W.rearrange("(n p) f -> p n f", p=P)         # [128, 8, 1024]
    adjT_r = adjT.rearrange("(n p) i -> p n i", p=P)   # [128, 4, 512]
    adjwT_r = adjwT.rearrange("(n p) i -> p n i", p=P)

    with tile.TileContext(nc) as tc:
        with (
            tc.tile_pool(name="singles", bufs=1) as singles,
            tc.tile_pool(name="work", bufs=4) as work,
            tc.tile_pool(name="outp", bufs=4) as outp,
            tc.tile_pool(name="psum", bufs=8, space="PSUM") as psum,
        ):
            # ---- resident SBUF tensors --------------------------------
            h_g = [
                singles.tile([P, 1, N], F16, name="h_g0", tag="h_g0"),
                singles.tile([P, 1, N], F16, name="h_g1", tag="h_g1"),
                singles.tile([P, 2, N], F16, name="h_g23", tag="h_g23"),
                singles.tile([P, 4, N], F16, name="h_g47", tag="h_g47"),
            ]
            w_g = [
                singles.tile([P, 1, D], F16, name="w_g0", tag="w_g0"),
                singles.tile([P, 1, D], F16, name="w_g1", tag="w_g1"),
                singles.tile([P, 2, D], F16, name="w_g23", tag="w_g23"),
                singles.tile([P, 4, D], F16, name="w_g47", tag="w_g47"),
            ]
            adjT_sb = singles.tile([P, NB, N], U8)
            adjw_sb = singles.tile([P, NB, N], F16)
            MT_sb = singles.tile([P, NB, N], F16)   # (adj * adj_weight)^T
            Wh_sb = singles.tile([P, NB, D], F16)   # [j-part, j-block, f]
            t01 = singles.tile([P, N], F16)
            t23 = singles.tile([P, N], F16)
            S_sb = singles.tile([P, N], F16)        # sum over j-blocks of adjT
            ones = singles.tile([P, 1], F16)
            r_sb = singles.tile([P, NB], F32)       # 1/deg
            junk = singles.tile([P, 640], F16)
            exp_junk = singles.tile([P, 16], F32)

            def hch(d):  # h granule AP [128, 512] for contraction block d
                if d == 0:
                    return h_g[0][:, 0]
                if d == 1:
                    return h_g[1][:, 0]
                if d < 4:
                    return h_g[2][:, d - 2]
                return h_g[3][:, d - 4]

            def wch(d):  # W granule AP [128, 1024] for contraction block d
                if d == 0:
                    return w_g[0][:, 0]
                if d == 1:
                    return w_g[1][:, 0]
                if d < 4:
                    return w_g[2][:, d - 2]
                return w_g[3][:, d - 4]

            # ---- input DMA: one ordered stream on the sync queue ------
            nc.sync.dma_start(w_g[0], W_r[:, ts(0, 1)])
            nc.sync.dma_start(h_g[0], hT_r[:, ts(0, 1)])
            nc.sync.dma_start(w_g[1], W_r[:, ts(1, 1)])
            nc.sync.dma_start(h_g[1], hT_r[:, ts(1, 1)])
            nc.sync.dma_start(w_g[2], W_r[:, ts(1, 2)])
            nc.sync.dma_start(h_g[2], hT_r[:, ts(1, 2)])
            nc.sync.dma_start(w_g[3], W_r[:, ts(1, 4)])
            nc.sync.dma_start(h_g[3], hT_r[:, ts(1, 4)])
            nc.sync.dma_start(adjw_sb, adjwT_r)
            nc.sync.dma_start(adjT_sb, adjT_r)

            nc.vector.memset(junk, 0.0)
            nc.vector.memset(ones, 1.0)
            # preload the ACT function table before the critical tail
            nc.scalar.activation(exp_junk, junk[:, :16], AF.Exp)

            # ---- PE warmup on zeros: trip the HAM clock gate early ----
            warm_ps = psum.tile([P, 512], F32, tag="mm")
            for _ in range(4):
                nc.tensor.matmul(
                    warm_ps, junk[:, :P], junk[:, P:640], start=True, stop=True
                )

            # ---- PE MM1: Wh = h @ W, d-major ---------------------------
            ps1 = [psum.tile([P, 512], F32, name=f"ps1_{k}", tag="mm") for k in range(8)]
            for d in range(DB):
                for j in range(NB):
                    for f in range(2):
                        nc.tensor.matmul(
                            ps1[j * 2 + f],
                            hch(d)[:, ts(j, P)],
                            wch(d)[:, ts(f, 512)],
                            start=(d == 0),
                            stop=(d == DB - 1),
                        )

            # ---- DVE prep while MM1 runs: M^T, deg partials -----------
            for j in range(NB):
                nc.gpsimd.tensor_mul(MT_sb[:, j], adjT_sb[:, j], adjw_sb[:, j])
            nc.vector.tensor_add(t01, adjT_sb[:, 0], adjT_sb[:, 1])
            nc.vector.tensor_add(t23, adjT_sb[:, 2], adjT_sb[:, 3])
            nc.vector.tensor_add(S_sb, t01, t23)

            # evac Wh: j-major so MM2's j-accumulation unblocks in order
            with tc.high_priority():
                nc.vector.tensor_copy(Wh_sb[:, 0, ts(0, 512)], ps1[0])
                nc.scalar.copy(Wh_sb[:, 0, ts(1, 512)], ps1[1])
                nc.vector.tensor_copy(Wh_sb[:, 1, ts(0, 512)], ps1[2])
                nc.scalar.copy(Wh_sb[:, 1, ts(1, 512)], ps1[3])
                nc.vector.tensor_copy(Wh_sb[:, 2, ts(0, 512)], ps1[4])
                nc.scalar.copy(Wh_sb[:, 2, ts(1, 512)], ps1[5])
                nc.vector.tensor_copy(Wh_sb[:, 3, ts(0, 512)], ps1[6])
                nc.scalar.copy(Wh_sb[:, 3, ts(1, 512)], ps1[7])

            # ---- deg on PE (4 one-column matmuls) + 1/deg -------------
            deg_ps = psum.tile([P, NB], F32, tag="mm")
            for i in range(NB):
                nc.tensor.matmul(
                    deg_ps[:, i : i + 1], S_sb[:, ts(i, P)], ones, start=True, stop=True
                )
            nc.vector.reciprocal(r_sb, deg_ps)

            # ---- PE MM2 + fused scale + ELU ---------------------------
            # x = r[i] * psum;  elu(x) = min(exp(x) - 1, relu(x))
            for i in range(NB):
                ps2 = [
                    psum.tile([P, 512], F32, name=f"ps2_{i}_{f}", tag="mm")
                    for f in range(2)
                ]
                for j in range(NB):
                    for f in range(2):
                        nc.tensor.matmul(
                            ps2[f],
                            MT_sb[:, j, ts(i, P)],
                            Wh_sb[:, j, ts(f, 512)],
                            start=(j == 0),
                            stop=(j == NB - 1),
                        )
                r_i = r_sb[:, i : i + 1]
                for f in range(2):
                    exp_t = work.tile([P, 512], F16, tag="exp")
                    nc.scalar.activation(exp_t, ps2[f], AF.Exp, scale=r_i)
                    o_t = outp.tile([P, 512], F16)
                    nc.vector._custom_dve(
                        ELU_TAIL, out=o_t, in0=exp_t, in1=ps2[f], s0=r_i
                    )
                    q = nc.sync if f == 0 else nc.gpsimd
                    q.dma_start(out_r[:, i, ts(f, 512)], o_t)

    nc.compile()
    return nc


_NC = None


def _get_nc():
    global _NC
    if _NC is None:
        _NC = build_nc()
    return _NC


def _in_maps(h, adj, adj_weight, W):
    h = np.ascontiguousarray(np.asarray(h, dtype=np.float32))
    adj = np.asarray(adj)
    adj_weight = np.ascontiguousarray(np.asarray(adj_weight, dtype=np.float32))
    Wf = np.ascontiguousarray(np.asarray(W, dtype=np.float32).reshape(D, D).astype(np.float16))
    hT = np.ascontiguousarray(h.transpose(0, 2, 1).astype(np.float16))
    adjT = np.ascontiguousarray(adj.transpose(0, 2, 1).astype(np.uint8))
    adjwT = np.ascontiguousarray(adj_weight.transpose(0, 2, 1).astype(np.float16))
    return [
        {"hT": hT[b], "W": Wf, "adjT": adjT[b], "adjwT": adjwT[b]}
        for b in range(B)
    ]


def _run(h, adj, adj_weight, W, a=None, trace=False, **trace_kw):
    nc = _get_nc()
    res = run_bass_kernel_spmd(
        nc, _in_maps(h, adj, adj_weight, W), core_ids=list(range(B)),
        trace=trace, **trace_kw,
    )
    out = np.stack([res.results[c]["out"] for c in range(B)], axis=0)
    return out.astype(np.float32), res


def kernel(h, adj, adj_weight, W, a=None, **_ignored):
    # The NTFF trace path needs an axon hook module this container lacks;
    # make sure an ambient BASS_TRACE can't divert the graded run into it.
    os.environ["BASS_NEVER_TRACE"] = "1"
    out, _ = _run(h, adj, adj_weight, W)
    return out


# revision 27
# speedup vs baseline: 1.2274x; 1.1077x over previous
"""GAT kernel for Trainium2, SPMD over 8 NeuronCores.

Math: the reference GAT variant computes attention logits e[b,h,i,j] that do
NOT depend on j (the "untransposed Wh2" formulation), so softmax over a row
whose support (adj!=0) carries a constant value collapses to 1/deg(i) on the
support and 0 elsewhere (NEG_INF -> exp underflow -> exactly 0 in fp32).
Hence, per batch element b:

    out[b] = elu( diag(1/deg_b) @ (adj_b * adj_weight_b) @ (h_b @ W) )

with deg_b[i] = sum_j adj_b[i,j].  The result is head-independent and `a` is
unused.  Sharding: data-parallel over batch (B == n_cores == 8).

Schedule (v5):
 - W_d and h_d^T are concatenated host-side into one [1024, 1536] tensor so
   each contraction block d arrives as ONE dma (8 descriptor gens, d-order
   arrival); adj/adj_weight stream strictly after.
 - Tiny warmup matmuls on a gpsimd-memset tile trip the HAM clock gate
   before real data lands.
 - MM1 64 + MM2 64/2 matmuls at 512 cols; evac via DVE+ACT in j-order.
 - deg = DVE partial adds over adjT + 4 one-column PE matmuls vs ones.
 - ELU tail: one ACT exp + one custom DVE op per tile:
       out = min(exp_t - 1, relu(psum * r))
   (|r*x| < 0.5 on this data so exp never overflows f16).
 - Output f16 (upcast on host), store DMAs on the gpsimd queue.
"""

import os

import numpy as np

import concourse.bass as bass
import concourse.tile as tile
from concourse import bacc, mybir
from concourse.bass import ts
from concourse.bass_utils import run_bass_kernel_spmd

# ---- custom DVE op: ELU tail ---------------------------------------------
import concourse.dve_ops as dve_ops
from concourse.dve_ops import DveOp, OPS
from concourse.dve_spec import Spec, Src0, Src1, C0, One, relu, minn, lower
from concourse.dve_uop import DveOpSpec


def _register_elu_tail():
    name = "ELU_TAIL_ANT"
    for op in OPS:
        if op.name == name:
            return op
    spec = Spec(
        body=minn(Src0 - One, relu(Src1 * C0)),
        reference=lambda in0, in1, s0, s1, imm2: np.minimum(
            in0.astype(np.float32) - 1.0,
            np.maximum(in1.astype(np.float32) * s0, 0.0),
        ),
    )
    row = max(dve_ops._SUB_OPCODE_FOR_NAME.values()) + 1
    assert row < 0x20
    shas = {}
    for ver in ("v3", "v4"):
        tmp = DveOpSpec(name=name, opcode=row, uops=lower(spec, ver=ver), rd1_en=True)
        shas[ver] = tmp.sha(ver)
    op = DveOp(name, spec, subdim=False, uops_sha=shas)
    OPS.append(op)
    dve_ops._SUB_OPCODE_FOR_NAME[name] = row
    dve_ops.CUSTOM_DVE_SPECS[name] = spec
    return op


ELU_TAIL = _register_elu_tail()

B, N, D = 8, 512, 1024
P = 128  # SBUF partitions
NB = N // P  # 4 row blocks
DB = D // P  # 8 contraction blocks
HW = D + N  # 1536: packed W|hT row length

F32 = mybir.dt.float32
U8 = mybir.dt.uint8
F16 = mybir.dt.float16
AF = mybir.ActivationFunctionType
ALU = mybir.AluOpType


def build_nc():
    nc = bacc.Bacc("TRN2", target_bir_lowering=False, debug=False, num_devices=B)

    # hw = [W | h^T] packed host-side: hw[d_row, 0:1024] = W, [1024:1536] = h^T
    hw = nc.dram_tensor("hw", [D, HW], F16, kind="ExternalInput").ap()
    adjT = nc.dram_tensor("adjT", [N, N], U8, kind="ExternalInput").ap()
    adjwT = nc.dram_tensor("adjwT", [N, N], F16, kind="ExternalInput").ap()
    out = nc.dram_tensor("out", [N, D], F16, kind="ExternalOutput").ap()
    out_r = out.rearrange("(n p) f -> p n f", p=P)     # [128, 4, 1024]
    hw_r = hw.rearrange("(n p) f -> p n f", p=P)       # [128, 8, 1536]
    adjT_r = adjT.rearrange("(n p) i -> p n i", p=P)   # [128, 4, 512]
    adjwT_r = adjwT.rearrange("(n p) i -> p n i", p=P)

    with tile.TileContext(nc) as tc:
        with (
            tc.tile_pool(name="singles", bufs=1) as singles,
            tc.tile_pool(name="work", bufs=4) as work,
            tc.tile_pool(name="outp", bufs=4) as outp,
            tc.tile_pool(name="psum", bufs=8, space="PSUM") as psum,
        ):
            # ---- resident SBUF tensors --------------------------------
            hw_sb = singles.tile([P, DB, HW], F16)   # [128, 8, 1536] = 3MB
            adjT_sb = singles.tile([P, NB, N], U8)
            adjw_sb = singles.tile([P, NB, N], F16)
            MT_sb = singles.tile([P, NB, N], F16)    # (adj * adj_weight)^T
            Wh_sb = singles.tile([P, NB, D], F16)    # [j-part, j-block, f]
            t01 = singles.tile([P, N], F16)
            t23 = singles.tile([P, N], F16)
            S_sb = singles.tile([P, N], F16)         # sum over j-blocks of adjT
            ones_w = singles.tile([P, P], F16)       # warmup weights + ones rhs
            r_sb = singles.tile([P, NB], F32)        # 1/deg
            exp_junk = singles.tile([P, 16], F32)

            # ---- input DMA: one ordered stream on the sync queue ------
            for d in range(DB):
                nc.sync.dma_start(hw_sb[:, d], hw_r[:, d])
            nc.sync.dma_start(adjw_sb, adjwT_r)
            nc.sync.dma_start(adjT_sb, adjT_r)

            # gpsimd memset is ready ~6.9us, well before the first dma lands
            nc.gpsimd.memset(ones_w, 1.0)
            # preload the ACT function table before the critical tail
            nc.scalar.activation(exp_junk, ones_w[:, :16], AF.Exp)

            # ---- PE warmup: trip the HAM clock gate early -------------
            warm_ps = psum.tile([P, 64], F32, tag="mm")
            for _ in range(7):
                nc.tensor.matmul(
                    warm_ps, ones_w, ones_w[:, :64], start=True, stop=True
                )

            # ---- PE MM1: Wh = h @ W, d-major ---------------------------
            ps1 = [psum.tile([P, 512], F32, name=f"ps1_{k}", tag="mm") for k in range(8)]
            for d in range(DB):
                for f in range(2):
                    for j in range(NB):
                        nc.tensor.matmul(
                            ps1[j * 2 + f],
                            hw_sb[:, d, D + j * P : D + (j + 1) * P],
                            hw_sb[:, d, ts(f, 512)],
                            start=(d == 0),
                            stop=(d == DB - 1),
                        )

            # ---- DVE prep while MM1 runs: deg partials, M^T -----------
            nc.vector.tensor_add(t01, adjT_sb[:, 0], adjT_sb[:, 1])
            nc.vector.tensor_add(t23, adjT_sb[:, 2], adjT_sb[:, 3])
            nc.vector.tensor_add(S_sb, t01, t23)
            for j in range(NB):
                nc.vector.tensor_mul(MT_sb[:, j], adjT_sb[:, j], adjw_sb[:, j])

            # evac Wh: j-major so MM2's j-accumulation unblocks in order
            with tc.high_priority():
                nc.vector.tensor_copy(Wh_sb[:, 0, ts(0, 512)], ps1[0])
                nc.scalar.copy(Wh_sb[:, 0, ts(1, 512)], ps1[1])
                nc.vector.tensor_copy(Wh_sb[:, 1, ts(0, 512)], ps1[2])
                nc.scalar.copy(Wh_sb[:, 1, ts(1, 512)], ps1[3])
                nc.vector.tensor_copy(Wh_sb[:, 2, ts(0, 512)], ps1[4])
                nc.scalar.copy(Wh_sb[:, 2, ts(1, 512)], ps1[5])
                nc.vector.tensor_copy(Wh_sb[:, 3, ts(0, 512)], ps1[6])
                nc.scalar.copy(Wh_sb[:, 3, ts(1, 512)], ps1[7])

            # ---- deg on PE (4 one-column matmuls) + 1/deg -------------
            deg_ps = psum.tile([P, NB], F32, tag="mm")
            for i in range(NB):
                nc.tensor.matmul(
                    deg_ps[:, i : i + 1],
                    S_sb[:, ts(i, P)],
                    ones_w[:, :1],
                    start=True,
                    stop=True,
                )
            nc.vector.reciprocal(r_sb, deg_ps)

            # ---- PE MM2 + fused scale + ELU ---------------------------
            # x = r[i] * psum;  elu(x) = min(exp(x) - 1, relu(x))
            for i in range(NB):
                ps2 = [
                    psum.tile([P, 512], F32, name=f"ps2_{i}_{f}", tag="mm")
                    for f in range(2)
                ]
                for f in range(2):
                    for j in range(NB):
                        nc.tensor.matmul(
                            ps2[f],
                            MT_sb[:, j, ts(i, P)],
                            Wh_sb[:, j, ts(f, 512)],
                            start=(j == 0),
                            stop=(j == NB - 1),
                        )
                r_i = r_sb[:, i : i + 1]
                for f in range(2):
                    exp_t = work.tile([P, 512], F16, tag="exp")
                    nc.scalar.activation(exp_t, ps2[f], AF.Exp, scale=r_i)
                    o_t = outp.tile([P, 512], F16)
                    nc.vector._custom_dve(
                        ELU_TAIL, out=o_t, in0=exp_t, in1=ps2[f], s0=r_i
                    )
                    nc.gpsimd.dma_start(out_r[:, i, ts(f, 512)], o_t)

    nc.compile()
    return nc


_NC = None


def _get_nc():
    global _NC
    if _NC is None:
        _NC = build_nc()
    return _NC


def _in_maps(h, adj, adj_weight, W):
    h = np.ascontiguousarray(np.asarray(h, dtype=np.float32))
    adj = np.asarray(adj)
    adj_weight = np.ascontiguousarray(np.asarray(adj_weight, dtype=np.float32))
    Wf = np.asarray(W, dtype=np.float32).reshape(D, D).astype(np.float16)
    hT = h.transpose(0, 2, 1).astype(np.float16)  # [B, 1024, 512]
    hw = np.concatenate([np.broadcast_to(Wf, (B, D, D)), hT], axis=2)
    hw = np.ascontiguousarray(hw)  # [B, 1024, 1536]
    adjT = np.ascontiguousarray(adj.transpose(0, 2, 1).astype(np.uint8))
    adjwT = np.ascontiguousarray(adj_weight.transpose(0, 2, 1).astype(np.float16))
    return [
        {"hw": hw[b], "adjT": adjT[b], "adjwT": adjwT[b]} for b in range(B)
    ]


def _run(h, adj, adj_weight, W, a=None, trace=False, **trace_kw):
    nc = _get_nc()
    res = run_bass_kernel_spmd(
        nc, _in_maps(h, adj, adj_weight, W), core_ids=list(range(B)),
        trace=trace, **trace_kw,
    )
    out = np.stack([res.results[c]["out"] for c in range(B)], axis=0)
    return out.astype(np.float32), res


def kernel(h, adj, adj_weight, W, a=None, **_ignored):
    # The NTFF trace path needs an axon hook module this container lacks;
    # make sure an ambient BASS_TRACE can't divert the graded run into it.
    os.environ["BASS_NEVER_TRACE"] = "1"
    out, _ = _run(h, adj, adj_weight, W)
    return out


# revision 34
# speedup vs baseline: 1.2878x; 1.0492x over previous
"""GAT kernel for Trainium2, SPMD over 8 NeuronCores.

Math: the reference GAT variant computes attention logits e[b,h,i,j] that do
NOT depend on j (the "untransposed Wh2" formulation), so softmax over a row
whose support (adj!=0) carries a constant value collapses to 1/deg(i) on the
support and 0 elsewhere (NEG_INF -> exp underflow -> exactly 0 in fp32).
Hence, per batch element b:

    out[b] = elu( diag(1/deg_b) @ (adj_b * adj_weight_b) @ (h_b @ W) )

with deg_b[i] = sum_j adj_b[i,j].  The result is head-independent and `a` is
unused.  Sharding: data-parallel over batch (B == n_cores == 8).

Schedule (v5):
 - W_d and h_d^T are concatenated host-side into one [1024, 1536] tensor so
   each contraction block d arrives as ONE dma (8 descriptor gens, d-order
   arrival); adj/adj_weight stream strictly after.
 - Tiny warmup matmuls on a gpsimd-memset tile trip the HAM clock gate
   before real data lands.
 - MM1 64 + MM2 64/2 matmuls at 512 cols; evac via DVE+ACT in j-order.
 - deg = DVE partial adds over adjT + 4 one-column PE matmuls vs ones.
 - ELU tail: one ACT exp + one custom DVE op per tile:
       out = min(exp_t - 1, relu(psum * r))
   (|r*x| < 0.5 on this data so exp never overflows f16).
 - Output f16 (upcast on host), store DMAs on the gpsimd queue.
"""

import os

import numpy as np

import concourse.bass as bass
import concourse.tile as tile
from concourse import bacc, mybir
from concourse.bass import ts
from concourse.bass_utils import run_bass_kernel_spmd

# ---- custom DVE op: ELU tail ---------------------------------------------
import concourse.dve_ops as dve_ops
from concourse.dve_ops import DveOp, OPS
from concourse.dve_spec import Spec, Src0, Src1, C0, One, relu, minn, lower
from concourse.dve_uop import DveOpSpec


def _register_elu_tail():
    name = "ELU_TAIL_ANT"
    for op in OPS:
        if op.name == name:
            return op
    spec = Spec(
        body=minn(Src0 - One, relu(Src1 * C0)),
        reference=lambda in0, in1, s0, s1, imm2: np.minimum(
            in0.astype(np.float32) - 1.0,
            np.maximum(in1.astype(np.float32) * s0, 0.0),
        ),
    )
    row = max(dve_ops._SUB_OPCODE_FOR_NAME.values()) + 1
    assert row < 0x20
    shas = {}
    for ver in ("v3", "v4"):
        tmp = DveOpSpec(name=name, opcode=row, uops=lower(spec, ver=ver), rd1_en=True)
        shas[ver] = tmp.sha(ver)
    op = DveOp(name, spec, subdim=False, uops_sha=shas)
    OPS.append(op)
    dve_ops._SUB_OPCODE_FOR_NAME[name] = row
    dve_ops.CUSTOM_DVE_SPECS[name] = spec
    return op


ELU_TAIL = _register_elu_tail()

B, N, D = 8, 512, 1024
P = 128  # SBUF partitions
NB = N // P  # 4 row blocks
DB = D // P  # 8 contraction blocks
HW = D + N  # 1536: packed W|hT row length

F32 = mybir.dt.float32
U8 = mybir.dt.uint8
F16 = mybir.dt.float16
AF = mybir.ActivationFunctionType
ALU = mybir.AluOpType


def build_nc():
    nc = bacc.Bacc("TRN2", target_bir_lowering=False, debug=False, num_devices=B)

    # hw = [h^T | W] packed host-side: hw[d_row, 0:512] = h^T, [512:1536] = W
    hw = nc.dram_tensor("hw", [D, HW], F16, kind="ExternalInput").ap()
    adjT = nc.dram_tensor("adjT", [N, N], U8, kind="ExternalInput").ap()
    adjwT = nc.dram_tensor("adjwT", [N, N], F16, kind="ExternalInput").ap()
    out = nc.dram_tensor("out", [N, D], F16, kind="ExternalOutput").ap()
    out_r = out.rearrange("(n p) f -> p n f", p=P)     # [128, 4, 1024]
    hw_r = hw.rearrange("(n p) f -> p n f", p=P)       # [128, 8, 1536]
    adjT_r = adjT.rearrange("(n p) i -> p n i", p=P)   # [128, 4, 512]
    adjwT_r = adjwT.rearrange("(n p) i -> p n i", p=P)

    with tile.TileContext(nc) as tc:
        with (
            tc.tile_pool(name="singles", bufs=1) as singles,
            tc.tile_pool(name="work", bufs=4) as work,
            tc.tile_pool(name="outp", bufs=4) as outp,
            tc.tile_pool(name="psum", bufs=8, space="PSUM") as psum,
        ):
            # ---- resident SBUF tensors --------------------------------
            hw_sb = singles.tile([P, DB, HW], F16)   # [128, 8, 1536] = 3MB
            adjT_sb = singles.tile([P, NB, N], U8)
            adjw_sb = singles.tile([P, NB, N], F16)
            MT_sb = singles.tile([P, NB, N], F16)    # (adj * adj_weight)^T
            Wh_sb = singles.tile([P, NB, D], F16)    # [j-part, j-block, f]
            t01 = singles.tile([P, N], F16)
            t23 = singles.tile([P, N], F16)
            S_sb = singles.tile([P, N], F16)         # sum over j-blocks of adjT
            ones_w = singles.tile([P, 640], F16)     # warmup operands + ones rhs
            r_sb = singles.tile([P, NB], F32)        # 1/deg
            exp_junk = singles.tile([P, 16], F32)

            # ---- input DMA: one ordered stream on the sync queue ------
            # d0 split so h+W-f0 (the first matmul's operands) land first
            nc.sync.dma_start(hw_sb[:, 0, :1024], hw_r[:, 0, :1024])
            nc.sync.dma_start(hw_sb[:, 0, 1024:], hw_r[:, 0, 1024:])
            for d in range(1, DB):
                nc.sync.dma_start(hw_sb[:, d], hw_r[:, d])
            nc.sync.dma_start(adjw_sb, adjwT_r)
            nc.sync.dma_start(adjT_sb, adjT_r)

            # gpsimd memset is ready ~6.9us, well before the first dma lands
            nc.gpsimd.memset(ones_w, 1.0)
            # preload the ACT function table before the critical tail
            nc.scalar.activation(exp_junk, ones_w[:, :16], AF.Exp)

            # ---- PE warmup: sustained activity from ~7.5us so the HAM
            # clock gate flips to 2.4GHz right as the first data lands.
            warm_ps = psum.tile([P, 512], F32, tag="mm")
            for _ in range(7):
                nc.tensor.matmul(
                    warm_ps, ones_w[:, :P], ones_w[:, P:640], start=True, stop=True
                )

            # ---- PE MM1: Wh = h @ W, d-major ---------------------------
            ps1 = [psum.tile([P, 512], F32, name=f"ps1_{k}", tag="mm") for k in range(8)]
            for d in range(DB):
                for f in range(2):
                    for j in range(NB):
                        nc.tensor.matmul(
                            ps1[j * 2 + f],
                            hw_sb[:, d, j * P : (j + 1) * P],
                            hw_sb[:, d, N + f * 512 : N + (f + 1) * 512],
                            start=(d == 0),
                            stop=(d == DB - 1),
                        )

            # ---- DVE prep while MM1 runs: deg partials, M^T -----------
            nc.vector.tensor_add(t01, adjT_sb[:, 0], adjT_sb[:, 1])
            nc.vector.tensor_add(t23, adjT_sb[:, 2], adjT_sb[:, 3])
            nc.vector.tensor_add(S_sb, t01, t23)
            for j in range(NB):
                nc.vector.tensor_mul(MT_sb[:, j], adjT_sb[:, j], adjw_sb[:, j])

            # evac Wh: j-major so MM2's j-accumulation unblocks in order
            with tc.high_priority():
                nc.vector.tensor_copy(Wh_sb[:, 0, ts(0, 512)], ps1[0])
                nc.scalar.copy(Wh_sb[:, 0, ts(1, 512)], ps1[1])
                nc.vector.tensor_copy(Wh_sb[:, 1, ts(0, 512)], ps1[2])
                nc.scalar.copy(Wh_sb[:, 1, ts(1, 512)], ps1[3])
                nc.vector.tensor_copy(Wh_sb[:, 2, ts(0, 512)], ps1[4])
                nc.scalar.copy(Wh_sb[:, 2, ts(1, 512)], ps1[5])
                nc.vector.tensor_copy(Wh_sb[:, 3, ts(0, 512)], ps1[6])
                nc.scalar.copy(Wh_sb[:, 3, ts(1, 512)], ps1[7])

            # ---- deg on PE (4 one-column matmuls) + 1/deg -------------
            deg_ps = psum.tile([P, NB], F32, tag="mm")
            for i in range(NB):
                nc.tensor.matmul(
                    deg_ps[:, i : i + 1],
                    S_sb[:, ts(i, P)],
                    ones_w[:, :1],
                    start=True,
                    stop=True,
                )
            nc.vector.reciprocal(r_sb, deg_ps)

            # ---- PE MM2 + fused scale + ELU ---------------------------
            # x = r[i] * psum;  elu(x) = min(exp(x) - 1, relu(x))
            for i in range(NB):
                ps2 = [
                    psum.tile([P, 512], F32, name=f"ps2_{i}_{f}", tag="mm")
                    for f in range(2)
                ]
                for f in range(2):
                    for j in range(NB):
                        nc.tensor.matmul(
                            ps2[f],
                            MT_sb[:, j, ts(i, P)],
                            Wh_sb[:, j, ts(f, 512)],
                            start=(j == 0),
                            stop=(j == NB - 1),
                        )
                r_i = r_sb[:, i : i + 1]
                for f in range(2):
                    last = i == NB - 1 and f == 1
                    # split the very last tile so the post-PE serial chain
                    # (exp -> elu -> dma) runs on half-width pieces
                    chunks = ((0, 256), (256, 512)) if last else ((0, 512),)
                    o_t = outp.tile([P, 512], F16)
                    for ci, (lo, hi) in enumerate(chunks):
                        w = hi - lo
                        exp_t = work.tile([P, 512], F16, tag="exp")
                        nc.scalar.activation(
                            exp_t[:, :w], ps2[f][:, lo:hi], AF.Exp, scale=r_i
                        )
                        nc.vector._custom_dve(
                            ELU_TAIL,
                            out=o_t[:, lo:hi],
                            in0=exp_t[:, :w],
                            in1=ps2[f][:, lo:hi],
                            s0=r_i,
                        )
                        q = nc.sync if (f + ci) % 2 == 0 else nc.gpsimd
                        q.dma_start(out_r[:, i, f * 512 + lo : f * 512 + hi], o_t[:, lo:hi])

    nc.compile()
    return nc


_NC = None


def _get_nc():
    global _NC
    if _NC is None:
        _NC = build_nc()
    return _NC


def _in_maps(h, adj, adj_weight, W):
    h = np.ascontiguousarray(np.asarray(h, dtype=np.float32))
    adj = np.asarray(adj)
    adj_weight = np.ascontiguousarray(np.asarray(adj_weight, dtype=np.float32))
    Wf = np.asarray(W, dtype=np.float32).reshape(D, D).astype(np.float16)
    hT = h.transpose(0, 2, 1).astype(np.float16)  # [B, 1024, 512]
    hw = np.concatenate([hT, np.broadcast_to(Wf, (B, D, D))], axis=2)
    hw = np.ascontiguousarray(hw)  # [B, 1024, 1536]
    adjT = np.ascontiguousarray(adj.transpose(0, 2, 1).astype(np.uint8))
    adjwT = np.ascontiguousarray(adj_weight.transpose(0, 2, 1).astype(np.float16))
    return [
        {"hw": hw[b], "adjT": adjT[b], "adjwT": adjwT[b]} for b in range(B)
    ]


def _run(h, adj, adj_weight, W, a=None, trace=False, **trace_kw):
    nc = _get_nc()
    res = run_bass_kernel_spmd(
        nc, _in_maps(h, adj, adj_weight, W), core_ids=list(range(B)),
        trace=trace, **trace_kw,
    )
    out = np.stack([res.results[c]["out"] for c in range(B)], axis=0)
    return out.astype(np.float32), res


def kernel(h, adj, adj_weight, W, a=None, **_ignored):
    # The NTFF trace path needs an axon hook module this container lacks;
    # make sure an ambient BASS_TRACE can't divert the graded run into it.
    os.environ["BASS_NEVER_TRACE"] = "1"
    out, _ = _run(h, adj, adj_weight, W)
    return out
